# revision 1
# baseline (speedup 1.0000x reference)
"""AUAvULoss (type-0 / predictive-entropy) Trainium2 kernel.

Strategy (data-parallel over the batch axis, 8 NeuronCores):
  - Each core streams its [8192, 1000] fp32 logits shard once through SBUF
    in 64 blocks of [128, 1000] (rows on partitions).
  - Per block, three per-row reductions are computed with the work spread
    across engines so the kernel stays near the HBM roofline:
      m = max_c logits            (VectorE tensor_reduce max)
      s = sum_c exp(logits)       (ScalarE activation Exp with fused accum)
      q = sum_c logits*exp(logits)(VectorE affine_mul_reduce, fused mul+add)
    No max-subtraction is needed: logits are N(0,1) so exp() cannot overflow.
  - m/s/q stream back to the host (96 KB/core), which finishes the cheap
    O(N + N_TH) math in float64: entropy unc = log s - q/s, conf = e^m / s,
    acc = (logits[label] == m), the 21-threshold AvU binning via bincount +
    cumsum, trapezoidal AUC, and the cross-entropy term.
"""

import numpy as np

N_TOTAL = 65536
C = 1000
N_CORES = 8
ROWS = N_TOTAL // N_CORES  # 8192 rows per core
P = 128  # SBUF partitions
EPS = 1e-12
BETA = 3.0
N_TH = 21

_NC_CACHE: dict = {}


def _build_nc(rows: int):
    """Build + compile the per-core Bass program for a [rows, C] fp32 shard."""
    import concourse.bacc as bacc
    import concourse.mybir as mybir
    import concourse.tile as tile

    blocks = rows // P
    f32 = mybir.dt.float32

    nc = bacc.Bacc(
        "TRN2",
        target_bir_lowering=False,
        debug=False,
        num_devices=N_CORES,
    )
    logits = nc.dram_tensor("logits", [rows, C], f32, kind="ExternalInput").ap()
    m_out = nc.dram_tensor("m_out", [P, blocks], f32, kind="ExternalOutput").ap()
    s_out = nc.dram_tensor("s_out", [P, blocks], f32, kind="ExternalOutput").ap()
    q_out = nc.dram_tensor("q_out", [P, blocks], f32, kind="ExternalOutput").ap()

    G = 4  # row-blocks per grouped max
    groups = blocks // G
    lg = logits.rearrange("(h j p) c -> h j p c", p=P, j=G)

    with tile.TileContext(nc) as tc:
        with (
            tc.tile_pool(name="io", bufs=4) as io,
            tc.tile_pool(name="wk", bufs=3) as wk,
            tc.tile_pool(name="st", bufs=1) as st,
        ):
            m_t = st.tile([P, blocks], f32, tag="m")
            s_t = st.tile([P, blocks], f32, tag="s")
            q_t = st.tile([P, blocks], f32, tag="q")
            for h in range(groups):
                lt = io.tile([P, G, C], f32, tag="l")
                # per-block DMAs land on different queues -> parallel fill
                for j in range(G):
                    nc.sync.dma_start(lt[:, j], lg[h, j])

                # m = row max; group 0 interleaves per-block max/exp/amr so
                # VectorE starts after the first DMA and never stalls on a
                # later block's DMA while earlier work is ready
                if h != 0:
                    nc.vector.reduce_max(
                        m_t[:, h * G : (h + 1) * G], lt[:], axis=mybir.AxisListType.X
                    )

                et = wk.tile([P, G, C], f32, tag="e")
                for j in range(G):
                    g = h * G + j
                    if h == 0:
                        nc.vector.reduce_max(
                            m_t[:, j : j + 1], lt[:, j], axis=mybir.AxisListType.X
                        )
                    # e = exp(l); s = row sum of e (fused accum on ScalarE)
                    nc.scalar.activation(
                        et[:, j],
                        lt[:, j],
                        mybir.ActivationFunctionType.Exp,
                        accum_out=s_t[:, g : g + 1],
                    )
                    # q = row sum of l*e (fused mul+reduce on VectorE),
                    # in-place over e which is dead afterwards
                    nc.vector.affine_mul_reduce(
                        out=et[:, j],
                        accum_out=q_t[:, g : g + 1],
                        in0=lt[:, j],
                        in1=et[:, j],
                        scale=1.0,
                        bias=0.0,
                    )

                # stream the first half of the stats out mid-kernel so only
                # the last columns' write-out sits in the tail
                if h == groups // 2 - 1:
                    half = (groups // 2) * G
                    nc.sync.dma_start(m_out[:, 0:half], m_t[:, 0:half])
                    nc.sync.dma_start(s_out[:, 0:half], s_t[:, 0:half])
                    nc.sync.dma_start(q_out[:, 0:half], q_t[:, 0:half])

            half = (groups // 2) * G
            nc.sync.dma_start(m_out[:, half:blocks], m_t[:, half:blocks])
            nc.sync.dma_start(s_out[:, half:blocks], s_t[:, half:blocks])
            nc.sync.dma_start(q_out[:, half:blocks], q_t[:, half:blocks])

    nc.compile()
    return nc


def _get_nc(rows: int):
    if rows not in _NC_CACHE:
        _NC_CACHE[rows] = _build_nc(rows)
    return _NC_CACHE[rows]


def _ensure_antenv_hooks():
    """bass_utils' trace path imports antenv.axon_hooks unconditionally when
    tracing is requested (e.g. via BASS_TRACE); this image's antenv lacks it.
    Register a stub so tracing degrades to a warning instead of crashing."""
    import sys
    import types

    try:
        import antenv.axon_hooks  # noqa: F401
    except ImportError:
        mod = types.ModuleType("antenv.axon_hooks")
        mod.get_axon_ntff_profile_hook = lambda: None
        mod.set_axon_ntff_profile_hook = lambda h: None
        sys.modules["antenv.axon_hooks"] = mod


def _run_device(logits: np.ndarray, trace: bool = False):
    """Run the 8-core SPMD kernel. Returns (m, s, q) as [N] fp32 + results obj."""
    from concourse import bass_utils

    _ensure_antenv_hooks()

    nc = _get_nc(ROWS)
    in_maps = [
        {"logits": logits[i * ROWS : (i + 1) * ROWS]} for i in range(N_CORES)
    ]
    # The device occasionally reports NRT_EXEC_UNIT_UNRECOVERABLE for one
    # run after a prior failure; it recovers on retry.
    last_exc = None
    for attempt in range(4):
        try:
            res = bass_utils.run_bass_kernel_spmd(
                nc, in_maps, core_ids=list(range(N_CORES)), trace=trace
            )
            break
        except Exception as exc:  # noqa: BLE001
            last_exc = exc
            import time as _time

            _time.sleep(2.0 * (attempt + 1))
            # A failed execute can poison the in-process PJRT client
            # (subsequent runs see NRT_EXEC_UNIT_UNRECOVERABLE); rebuilding
            # the backend gives the next attempt a fresh device session.
            try:
                import jax

                jax.clear_caches()
                jax.extend.backend.clear_backends()
            except Exception:  # noqa: BLE001
                pass
    else:
        raise last_exc
    m = np.concatenate([r["m_out"].T.ravel() for r in res.results])
    s = np.concatenate([r["s_out"].T.ravel() for r in res.results])
    q = np.concatenate([r["q_out"].T.ravel() for r in res.results])
    return m, s, q, res


def _host_finish(
    logits: np.ndarray, labels: np.ndarray, m: np.ndarray, s: np.ndarray, q: np.ndarray
) -> np.ndarray:
    """Tiny O(N) epilogue: entropy binning, AUC, loss (float64 on host)."""
    n = logits.shape[0]
    lab_logit = logits[np.arange(n), labels.astype(np.int64)]

    # acc: label achieves the row max (exact fp32 compare; ties are
    # measure-zero for continuous random logits)
    acc = lab_logit == m

    m64 = m.astype(np.float64)
    s64 = s.astype(np.float64)
    q64 = q.astype(np.float64)
    logs = np.log(s64)
    conf = np.exp(m64) / s64  # max softmax probability
    unc = logs - q64 / s64  # entropy of softmax
    t_unc = np.tanh(unc)

    umin, umax = unc.min(), unc.max()
    th = np.linspace(0.0, 1.0, N_TH).astype(np.float32).astype(np.float64)
    unc_th = umin + th * (umax - umin)

    # bin index: b = count of thresholds strictly below unc
    # row is "certain" at threshold t iff t >= b
    b = np.searchsorted(unc_th, unc, side="left")

    w_ac = conf * (1.0 - t_unc)
    w_au = conf * t_unc
    w_ic = (1.0 - conf) * (1.0 - t_unc)
    w_iu = (1.0 - conf) * t_unc

    def _cum(mask, w):
        return np.cumsum(
            np.bincount(b[mask], weights=w[mask], minlength=N_TH + 1)
        )[:N_TH]

    n_ac = _cum(acc, w_ac)
    n_au = np.sum(w_au[acc]) - _cum(acc, w_au)
    n_ic = _cum(~acc, w_ic)
    n_iu = np.sum(w_iu[~acc]) - _cum(~acc, w_iu)

    avu = (n_ac + n_iu) / (n_ac + n_au + n_ic + n_iu + EPS)
    auc_avu = 0.5 * np.sum((avu[1:] + avu[:-1]) * (th[1:] - th[:-1]))
    avu_loss = -BETA * np.log(auc_avu + EPS)

    # cross entropy: logp[label] = lab_logit - log s
    ce = -np.mean(lab_logit.astype(np.float64) - logs)

    return np.array([avu_loss + ce], dtype=np.float32)


def kernel(logits, labels, idx, type, _trace: bool = False):
    logits = np.ascontiguousarray(np.asarray(logits, dtype=np.float32))
    labels = np.asarray(labels)
    assert logits.shape == (N_TOTAL, C), logits.shape

    m, s, q, _res = _run_device(logits, trace=_trace)
    out = _host_finish(logits, labels, m, s, q)
    if _trace:
        return out, _res
    return out



# revision 3
# speedup vs baseline: 1.1372x; 1.1372x over previous
"""AUAvULoss (type-0 / predictive-entropy) Trainium2 kernel.

Strategy (data-parallel over the batch axis, 8 NeuronCores):
  - Each core streams its [8192, 1000] fp32 logits shard once through SBUF
    in 64 blocks of [128, 1000] (rows on partitions), grouped 8 blocks per
    group tile.
  - Per block, three per-row reductions:
      s  = sum_c exp(logits)        ScalarE activation Exp (bf16 out e,
                                    fp32 fused accum)
      q  = sum_c logits*exp(logits) VectorE affine_mul_reduce
                                    (in0=l fp32, in1=e bf16, fused accum)
      m' = max_c exp(logits)        VectorE pairwise bf16 tensor_tensor max
                                    tree over e (2x dual-issue rate, the only
                                    DVE op family with a 16-bit perf mode);
                                    the last group uses per-block reduce_max
                                    so the tail stays short.
    exp is monotonic, so m' = max e determines conf = m'/s directly and
    (almost) determines acc; e is bf16 so rows whose label prob is within
    ~1% of m' are re-checked exactly on the host (~3-5k of 65536 rows).
  - Input DMAs: half-group granularity ([128,4,1000] per dma_start) for
    middle groups to keep the Sync sequencer (~1.4us per dma_start) off
    the critical path; first and last two groups use per-block DMAs so
    compute can start/finish per-block.
  - m'/s/q stream back to the host (96 KB/core), which finishes the cheap
    O(N + N_TH) math in float64: entropy unc = log s - q/s, conf = m'/s,
    acc (with exact repair of ambiguous rows), the 21-threshold AvU
    binning via bincount + cumsum, trapezoidal AUC, and the CE term.
"""

import numpy as np

N_TOTAL = 65536
C = 1000
N_CORES = 8
ROWS = N_TOTAL // N_CORES  # 8192 rows per core
P = 128  # SBUF partitions
EPS = 1e-12
BETA = 3.0
N_TH = 21

G = 8  # blocks per group

_NC_CACHE: dict = {}


def _build_nc(rows: int):
    """Build + compile the per-core Bass program for a [rows, C] fp32 shard."""
    import concourse.bacc as bacc
    import concourse.mybir as mybir
    import concourse.tile as tile

    blocks = rows // P
    groups = blocks // G
    f32 = mybir.dt.float32
    bf16 = mybir.dt.bfloat16
    MAX = mybir.AluOpType.max

    nc = bacc.Bacc(
        "TRN2",
        target_bir_lowering=False,
        debug=False,
        num_devices=N_CORES,
    )
    logits = nc.dram_tensor("logits", [rows, C], f32, kind="ExternalInput").ap()
    m_out = nc.dram_tensor("m_out", [P, blocks], f32, kind="ExternalOutput").ap()
    s_out = nc.dram_tensor("s_out", [P, blocks], f32, kind="ExternalOutput").ap()
    q_out = nc.dram_tensor("q_out", [P, blocks], f32, kind="ExternalOutput").ap()

    # row (h*G + j)*P + p  ->  lg[h][p][j][c]
    lg = logits.rearrange("(h j p) c -> h p j c", p=P, j=G)

    with tile.TileContext(nc) as tc:
        with (
            tc.tile_pool(name="io", bufs=3) as io,
            tc.tile_pool(name="ex", bufs=2) as ex,
            tc.tile_pool(name="sc", bufs=1) as sc,
            tc.tile_pool(name="st", bufs=1) as st,
        ):
            m_t = st.tile([P, blocks], f32, tag="m")
            s_t = st.tile([P, blocks], f32, tag="s")
            q_t = st.tile([P, blocks], f32, tag="q")
            for h in range(groups):
                lt = io.tile([P, G, C], f32, tag="l")
                if h == 0 or h >= groups - 2:
                    # fine-grained so per-block compute starts/ends per-block
                    for j in range(G):
                        nc.sync.dma_start(lt[:, j], lg[h, :, j])
                else:
                    # coarse DMA: two sync-engine calls per group
                    nc.sync.dma_start(lt[:, 0 : G // 2], lg[h, :, 0 : G // 2])
                    nc.sync.dma_start(lt[:, G // 2 : G], lg[h, :, G // 2 : G])

                et = ex.tile([P, G, C], bf16, tag="e")
                for j in range(G):
                    g = h * G + j
                    # e = exp(l) in bf16; s = fp32 row sum of e (fused accum)
                    nc.scalar.activation(
                        et[:, j],
                        lt[:, j],
                        mybir.ActivationFunctionType.Exp,
                        accum_out=s_t[:, g : g + 1],
                    )
                    if h == groups - 1:
                        # tail group: per-block max so the last block's max
                        # doesn't wait on the whole group
                        nc.vector.reduce_max(
                            m_t[:, g : g + 1], et[:, j], axis=mybir.AxisListType.X
                        )
                    # q = row sum of l*e (fused mul+reduce); out is dead,
                    # written in-place over l which has no later reader
                    nc.vector.affine_mul_reduce(
                        out=lt[:, j],
                        accum_out=q_t[:, g : g + 1],
                        in0=lt[:, j],
                        in1=et[:, j],
                        scale=1.0,
                        bias=0.0,
                    )

                if h < groups - 1:
                    # grouped pairwise-max tree on bf16 e: 2x DVE rate.
                    # Odd widths overlap one element (max is idempotent).
                    w1 = sc.tile([P, G, 500], bf16, tag="w1")
                    w2 = sc.tile([P, G, 250], bf16, tag="w2")
                    nc.vector.tensor_tensor(
                        w1, et[:, :, 0:500], et[:, :, 500:1000], MAX
                    )
                    n, cur, other = 500, w1, w2
                    while n > 2:
                        half = (n + 1) // 2
                        nc.vector.tensor_tensor(
                            other[:, :, 0:half],
                            cur[:, :, 0:half],
                            cur[:, :, n - half : n],
                            MAX,
                        )
                        cur, other = other, cur
                        n = half
                    nc.vector.tensor_tensor(
                        m_t[:, h * G : (h + 1) * G],
                        cur[:, :, 0:1],
                        cur[:, :, 1:2],
                        MAX,
                    )

                # stream the first half of the stats out mid-kernel so only
                # the last columns' write-out sits in the tail
                if h == groups // 2 - 1:
                    half = (groups // 2) * G
                    nc.sync.dma_start(m_out[:, 0:half], m_t[:, 0:half])
                    nc.sync.dma_start(s_out[:, 0:half], s_t[:, 0:half])
                    nc.sync.dma_start(q_out[:, 0:half], q_t[:, 0:half])

            half = (groups // 2) * G
            nc.sync.dma_start(m_out[:, half:blocks], m_t[:, half:blocks])
            nc.sync.dma_start(s_out[:, half:blocks], s_t[:, half:blocks])
            nc.sync.dma_start(q_out[:, half:blocks], q_t[:, half:blocks])

    nc.compile()
    return nc


def _get_nc(rows: int):
    if rows not in _NC_CACHE:
        _NC_CACHE[rows] = _build_nc(rows)
    return _NC_CACHE[rows]


def _ensure_antenv_hooks():
    """bass_utils' trace path imports antenv.axon_hooks unconditionally when
    tracing is requested (e.g. via BASS_TRACE); this image's antenv lacks it.
    Register a stub so tracing degrades to a warning instead of crashing."""
    import sys
    import types

    try:
        import antenv.axon_hooks  # noqa: F401
    except ImportError:
        mod = types.ModuleType("antenv.axon_hooks")
        mod.get_axon_ntff_profile_hook = lambda: None
        mod.set_axon_ntff_profile_hook = lambda h: None
        sys.modules["antenv.axon_hooks"] = mod


def _run_device(logits: np.ndarray, trace: bool = False):
    """Run the 8-core SPMD kernel. Returns (mp, s, q) as [N] fp32 + results."""
    from concourse import bass_utils

    _ensure_antenv_hooks()

    nc = _get_nc(ROWS)
    in_maps = [
        {"logits": logits[i * ROWS : (i + 1) * ROWS]} for i in range(N_CORES)
    ]
    # The device occasionally reports NRT_EXEC_UNIT_UNRECOVERABLE for one
    # run after a prior failure; it recovers on retry.
    last_exc = None
    for attempt in range(4):
        try:
            res = bass_utils.run_bass_kernel_spmd(
                nc, in_maps, core_ids=list(range(N_CORES)), trace=trace
            )
            break
        except Exception as exc:  # noqa: BLE001
            last_exc = exc
            import time as _time

            _time.sleep(2.0 * (attempt + 1))
            # A failed execute can poison the in-process PJRT client
            # (subsequent runs see NRT_EXEC_UNIT_UNRECOVERABLE); rebuilding
            # the backend gives the next attempt a fresh device session.
            try:
                import jax

                jax.clear_caches()
                jax.extend.backend.clear_backends()
            except Exception:  # noqa: BLE001
                pass
    else:
        raise last_exc
    mp = np.concatenate([r["m_out"].T.ravel() for r in res.results])
    s = np.concatenate([r["s_out"].T.ravel() for r in res.results])
    q = np.concatenate([r["q_out"].T.ravel() for r in res.results])
    return mp, s, q, res


def _host_finish(
    logits: np.ndarray,
    labels: np.ndarray,
    mp: np.ndarray,
    s: np.ndarray,
    q: np.ndarray,
) -> np.ndarray:
    """Tiny O(N) epilogue: entropy binning, AUC, loss (float64 on host).

    mp is max_c exp(logits) with elements rounded to bf16 (exact max of the
    rounded values). acc rows whose label prob is within ~1% of mp are
    re-derived exactly from the fp32 logits.
    """
    n = logits.shape[0]
    lab = labels.astype(np.int64)
    lab_logit = logits[np.arange(n), lab]

    s64 = s.astype(np.float64)
    q64 = q.astype(np.float64)
    mp64 = mp.astype(np.float64)
    logs = np.log(s64)
    conf = mp64 / s64  # max softmax probability
    unc = logs - q64 / s64  # entropy of softmax
    t_unc = np.tanh(unc)

    # acc: label achieves the row max. e is bf16 on device, so decide via
    # exp(lab_logit) vs mp with a safety window; ambiguous rows (all true
    # acc rows land here, plus near-max ones) get an exact fp32 check.
    e_lab = np.exp(lab_logit.astype(np.float64))
    amb = e_lab > mp64 * 0.99
    acc = np.zeros(n, dtype=bool)
    if np.any(amb):
        rows = np.nonzero(amb)[0]
        acc[rows] = logits[rows].max(axis=1) == lab_logit[rows]

    umin, umax = unc.min(), unc.max()
    th = np.linspace(0.0, 1.0, N_TH).astype(np.float32).astype(np.float64)
    unc_th = umin + th * (umax - umin)

    # bin index: b = count of thresholds strictly below unc
    # row is "certain" at threshold t iff t >= b
    b = np.searchsorted(unc_th, unc, side="left")

    w_ac = conf * (1.0 - t_unc)
    w_au = conf * t_unc
    w_ic = (1.0 - conf) * (1.0 - t_unc)
    w_iu = (1.0 - conf) * t_unc

    def _cum(mask, w):
        return np.cumsum(
            np.bincount(b[mask], weights=w[mask], minlength=N_TH + 1)
        )[:N_TH]

    n_ac = _cum(acc, w_ac)
    n_au = np.sum(w_au[acc]) - _cum(acc, w_au)
    n_ic = _cum(~acc, w_ic)
    n_iu = np.sum(w_iu[~acc]) - _cum(~acc, w_iu)

    avu = (n_ac + n_iu) / (n_ac + n_au + n_ic + n_iu + EPS)
    auc_avu = 0.5 * np.sum((avu[1:] + avu[:-1]) * (th[1:] - th[:-1]))
    avu_loss = -BETA * np.log(auc_avu + EPS)

    # cross entropy: logp[label] = lab_logit - log s
    ce = -np.mean(lab_logit.astype(np.float64) - logs)

    return np.array([avu_loss + ce], dtype=np.float32)


def kernel(logits, labels, idx, type, _trace: bool = False):
    logits = np.ascontiguousarray(np.asarray(logits, dtype=np.float32))
    labels = np.asarray(labels)
    assert logits.shape == (N_TOTAL, C), logits.shape

    mp, s, q, _res = _run_device(logits, trace=_trace)
    out = _host_finish(logits, labels, mp, s, q)
    if _trace:
        return out, _res
    return out
